# revision 17
# baseline (speedup 1.0000x reference)
"""Causal self-attention (RMSNorm-QK + RoPE) Trainium2 Bass kernel, v2.

Problem: B=2, T=2048, C=1024, H=16 heads, D=64.
Sharding: 8 cores = 2 (batch) x 4 (head groups of 4 heads). Host sums the
4 column-parallel out-proj partials per batch and transposes.

v2 design (vs baseline ~360us):
- bf16 everywhere except PSUM accum, den/invden math, and the final output.
- Fully fused single pass over 4 token blocks of 512: projections -> RMS ->
  RoPE -> attention(j=n) -> normalize+out-proj(n). No serial phases.
- Scores matmuls packed 2 heads/instruction via K=64 row-tiling
  (base_partition 0/64 auto tile_position).
- exp: one ACTIVATE per (pair, key-block) over a [128,2,512] PSUM pair tile.
- One manual ACT table load (natural_log_exp_and_others) serves the RMS
  Ln/Exp rsqrt and the attention Exp: no table thrashing.
- Causal masks: 4 precomputed bf16 [128,2,512] tiles, applied with DVE mult.
- Denominator via ones-column in v (M=65 AV); 1/den via DVE
  reciprocal_approx_fast on f32; bf16 den storage.
- Single big weight DMA + 3D-AP x loads to avoid startup DMA serialization.
"""

import sys

for _p in ("/opt/trn_rl_repo",):
    if _p not in sys.path:
        sys.path.append(_p)

import numpy as np

B, T, C = 2, 2048, 1024
H_TOT, D = 16, 64
HPC = 4               # heads per core
N_CORES = 8
P = 128
NB = 4                # token blocks
TB = 512              # token block size
KCH = 8               # C / 128 contraction chunks
RMS_EPS = 1.1920928955078125e-07
ROPE_BASE = 10000.0
ACT_SET_LN_EXP = 6    # natural_log_exp_and_others in act_info.json

_CACHE = {}


def _build_consts():
    import ml_dtypes
    bf = ml_dtypes.bfloat16
    inv_freq = (1.0 / (ROPE_BASE ** (np.arange(0, D, 2, dtype=np.float32) / np.float32(D)))).astype(np.float32)
    pos = np.arange(T, dtype=np.float32)
    freqs = np.outer(pos, inv_freq).astype(np.float32)      # [T, 32]
    cos = np.cos(freqs).astype(np.float32)
    sin = np.sin(freqs).astype(np.float32)
    cosr = np.ascontiguousarray(np.tile(cos.T, (HPC, 1))).astype(bf)   # [128, T]
    sinr = np.ascontiguousarray(np.tile(sin.T, (HPC, 1))).astype(bf)
    # ind32 [128, 4]: per-32-row-group summing matrix (lhsT for RMS sums)
    ind32 = np.zeros((P, HPC), dtype=np.float32)
    for p_ in range(P):
        ind32[p_, p_ // 32] = 1.0
    # bc32 [36, 128]: broadcast inv (4 heads) to 32-row groups (lhsT),
    # replicated at rows 0:4 (q) and 32:36 (k)
    bc32 = np.zeros((36, P), dtype=np.float32)
    for p_ in range(P):
        bc32[p_ // 32, p_] = 1.0
        bc32[32 + p_ // 32, p_] = 1.0
    # selpair [128, 256]: chunk c: col m -> den row 32*(2c + m//64)
    selpair = np.zeros((P, 2 * P), dtype=np.float32)
    for c in range(2):
        for m in range(P):
            selpair[32 * (2 * c + m // 64), 128 * c + m] = 1.0
    return dict(cosr=cosr, sinr=sinr, ind32=ind32.astype(bf),
                bc32=bc32.astype(bf), selpair=selpair.astype(bf))


def _build_module():
    import concourse.bacc as bacc
    import concourse.mybir as mybir
    import concourse.tile as tile

    f32 = mybir.dt.float32
    bf16 = mybir.dt.bfloat16
    Exp = mybir.ActivationFunctionType.Exp
    Ln = mybir.ActivationFunctionType.Ln
    Alu = mybir.AluOpType

    nc = bacc.Bacc("TRN2", target_bir_lowering=False, debug=False,
                   num_devices=N_CORES)

    # DRAM tensors. x is [128, 8, T] (chunk k at [:, k, :], row p = chan 128k+p)
    xt_d = nc.dram_tensor("xt", [P, KCH, T], bf16, kind="ExternalInput").ap()
    wq_d = nc.dram_tensor("wq", [P, KCH, 256], bf16, kind="ExternalInput").ap()
    wk_d = nc.dram_tensor("wk", [P, KCH, 256], bf16, kind="ExternalInput").ap()
    wv_d = nc.dram_tensor("wv", [P, KCH, 256], bf16, kind="ExternalInput").ap()
    wp_d = nc.dram_tensor("wp", [P, 2, C], bf16, kind="ExternalInput").ap()
    cosr_d = nc.dram_tensor("cosr", [P, T], bf16, kind="ExternalInput").ap()
    sinr_d = nc.dram_tensor("sinr", [P, T], bf16, kind="ExternalInput").ap()
    ind32_d = nc.dram_tensor("ind32", [P, HPC], bf16, kind="ExternalInput").ap()
    bc32_d = nc.dram_tensor("bc32", [36, P], bf16, kind="ExternalInput").ap()
    selpair_d = nc.dram_tensor("selpair", [P, 2 * P], bf16, kind="ExternalInput").ap()
    # out rows permuted: [p, o, j] = orig row 256*o + 128*j + p (host undoes)
    out_d = nc.dram_tensor("outT", [P, 4, 2, T], f32, kind="ExternalOutput").ap()

    with tile.TileContext(nc) as tc:
        nc.scalar.add_instruction(mybir.InstLoadActFuncSet(
            name=nc.get_next_instruction_name(),
            act_func_set_id=ACT_SET_LN_EXP, ins=[], outs=[]))

        with (
            tc.tile_pool(name="sb", bufs=1) as sb,
            tc.tile_pool(name="tr", bufs=2) as tr,
            tc.tile_pool(name="ps", bufs=1, space="PSUM") as ps,
        ):
            # ---- persistent SBUF ----
            def load(name, dram, shape, dt=bf16):
                t = sb.tile(shape, dt, tag=name, name=name)
                nc.sync.dma_start(out=t[:], in_=dram)
                return t

            # load order = need order: wq/wk first, wp last
            wq_a = load("wq_a", wq_d[:, 0:4, :], [P, 4, 256])
            wq_b = load("wq_b", wq_d[:, 4:8, :], [P, 4, 256])
            wk_t = load("wk", wk_d[:, :, :], [P, KCH, 256])
            wv_t = load("wv", wv_d[:, :, :], [P, KCH, 256])
            ind32_t = load("ind32", ind32_d[:, :], [P, HPC])
            bc32_t = load("bc32", bc32_d[:, :], [36, P])
            cosr_t = load("cosr", cosr_d[:, :], [P, T])
            sinr_t = load("sinr", sinr_d[:, :], [P, T])
            selpair_t = load("selpair", selpair_d[:, :], [P, 2 * P])
            wp_t = load("wp", wp_d[:, :, :], [P, 2, C])

            qT = [sb.tile([P, T], bf16, tag=f"qT{c}", name=f"qT{c}")
                  for c in range(2)]
            kT = [sb.tile([P, T], bf16, tag=f"kT{c}", name=f"kT{c}")
                  for c in range(2)]
            v_r = [sb.tile([P, HPC, 65], bf16, tag=f"v{s}", name=f"v{s}")
                   for s in range(T // P)]
            for s in range(T // P):
                nc.gpsimd.memset(v_r[s][:, :, 64:65], 1.0)
            yT = [sb.tile([P, T], bf16, tag=f"yT{c}", name=f"yT{c}")
                  for c in range(2)]
            den_bf = sb.tile([P, T], bf16, tag="denbf", name="den_bf")
            nc.gpsimd.memset(den_bf[:], 1.0)

            eps_t = sb.tile([36, 1], f32, tag="epst", name="eps_t")
            nc.gpsimd.memset(eps_t[:], RMS_EPS)

            invden_f = [sb.tile([P, TB], f32, tag="invdf", name=f"invdf{n}",
                                bufs=2) for n in range(NB)]
            invden_b = [sb.tile([P, TB], bf16, tag="invdb", name=f"invdb{n}",
                                bufs=2) for n in range(NB)]

            def rope_one(eng, x_pair, invb, rc1, rc2, nsl, nm):
                """rc1 = (x1 cos + x2 sin) inv ; rc2 = (x2 cos - x1 sin) inv."""
                x1 = x_pair[:, 0, :]
                x2 = x_pair[:, 1, :]
                ma = tr.tile([P, TB], bf16, tag=f"ma{nm}", name=f"ma{nm}", bufs=2)
                mb = tr.tile([P, TB], bf16, tag=f"mb{nm}", name=f"mb{nm}", bufs=2)
                eng.tensor_mul(ma[:], x1, cosr_t[:, nsl])
                eng.tensor_mul(mb[:], x2, sinr_t[:, nsl])
                eng.tensor_add(ma[:], ma[:], mb[:])
                eng.tensor_mul(rc1[:], ma[:], invb[:])
                mc = tr.tile([P, TB], bf16, tag=f"mc{nm}", name=f"mc{nm}", bufs=2)
                md = tr.tile([P, TB], bf16, tag=f"md{nm}", name=f"md{nm}", bufs=2)
                eng.tensor_mul(mc[:], x2, cosr_t[:, nsl])
                eng.tensor_mul(md[:], x1, sinr_t[:, nsl])
                eng.tensor_sub(mc[:], mc[:], md[:])
                eng.tensor_mul(rc2[:], mc[:], invb[:])

            def p1_block(n):
                """Projections + RMS inv + RoPE + repack + v for block n."""
                nsl = slice(n * TB, (n + 1) * TB)
                xr_a = tr.tile([P, 4, TB], bf16, tag="xra", name=f"xra{n}", bufs=2)
                nc.sync.dma_start(out=xr_a[:], in_=xt_d[:, 0:4, nsl])
                xr_b = tr.tile([P, 4, TB], bf16, tag="xrb", name=f"xrb{n}", bufs=2)
                nc.sync.dma_start(out=xr_b[:], in_=xt_d[:, 4:8, nsl])

                def xrc(k):
                    return (xr_a if k < 4 else xr_b)[:, k % 4, :]

                # ---- q/k projections + squares ----
                xps, sqs = {}, {}
                for tens, w_ts in (("q", (wq_a, wq_b)), ("k", (wk_t,))):
                    xp = tr.tile([P, 2, TB], bf16, tag="xp", name=f"xp{tens}{n}",
                                 bufs=2)
                    for half in range(2):
                        pg = ps.tile([P, TB], f32, tag="WK", name=f"p{tens}{half}_{n}",
                                     bufs=2)
                        for k in range(KCH):
                            if len(w_ts) == 2:
                                wsl = w_ts[k // 4][:, k % 4, 128 * half:128 * half + 128]
                            else:
                                wsl = w_ts[0][:, k, 128 * half:128 * half + 128]
                            nc.tensor.matmul(
                                pg[:], lhsT=wsl, rhs=xrc(k),
                                start=(k == 0), stop=(k == KCH - 1))
                        nc.vector.tensor_copy(xp[:, half, :], pg[:])
                    sq = tr.tile([P, 2, TB], bf16, tag="sq", name=f"sq{tens}{n}",
                                 bufs=2)
                    nc.vector.tensor_mul(sq[:], xp[:], xp[:])
                    xps[tens], sqs[tens] = xp, sq
                # ---- merged RMS inv: q at rows 0:4, k at rows 32:36 ----
                msp = ps.tile([36, TB], f32, tag="WK", name=f"ms{n}", bufs=2)
                for tens, rb in (("q", 0), ("k", 32)):
                    nc.tensor.matmul(msp[rb:rb + 4, :], lhsT=ind32_t[:],
                                     rhs=sqs[tens][:, 0, :], start=True, stop=False)
                    nc.tensor.matmul(msp[rb:rb + 4, :], lhsT=ind32_t[:],
                                     rhs=sqs[tens][:, 1, :], start=False, stop=True)
                invc = tr.tile([36, TB], bf16, tag="invc", name=f"invc{n}",
                               bufs=2)
                nc.scalar.activation(invc[:], msp[:], Ln,
                                     bias=eps_t[:], scale=1.0 / 64.0)
                nc.scalar.activation(invc[:], invc[:], Exp, scale=-0.5)
                # ---- RoPE + repack per tensor ----
                for tens, dstT, rb in (("q", qT, 0), ("k", kT, 32)):
                    invp = ps.tile([P, TB], f32, tag="WK", name=f"invp{tens}{n}",
                                   bufs=2)
                    nc.tensor.matmul(invp[:], lhsT=bc32_t[rb:rb + 4, :],
                                     rhs=invc[rb:rb + 4, :], start=True, stop=True)
                    invb = tr.tile([P, TB], bf16, tag="invb", name=f"invb{tens}{n}",
                                   bufs=2)
                    nc.vector.tensor_copy(invb[:], invp[:])
                    rc1 = tr.tile([P, TB], bf16, tag=f"rc1{tens}", name=f"rc1{tens}{n}",
                                  bufs=2)
                    rc2 = tr.tile([P, TB], bf16, tag=f"rc2{tens}", name=f"rc2{tens}{n}",
                                  bufs=2)
                    eng = nc.vector if tens == "q" else nc.gpsimd
                    rope_one(eng, xps[tens], invb, rc1, rc2, nsl, tens + str(n))
                    # repack into qT/kT: head h dims = rows 32h of rc1|rc2.
                    dma_eng = nc.gpsimd if tens == "q" else nc.sync
                    for c in range(2):
                        dst = dstT[c]
                        for h2, src in ((0, rc1), (1, rc2)):
                            dma_eng.dma_start(
                                out=dst[32 * h2:32 * h2 + 32, nsl],
                                in_=src[64 * c:64 * c + 32, :])
                            dma_eng.dma_start(
                                out=dst[64 + 32 * h2:64 + 32 * h2 + 32, nsl],
                                in_=src[64 * c + 32:64 * c + 64, :])

                # ---- v projection ----
                for s_rel in range(4):
                    pv = ps.tile([P, HPC, 64], f32, tag="WK", name=f"pv{n}_{s_rel}",
                                 bufs=2)
                    for k in range(KCH):
                        xt_sl = (xr_a if k < 4 else xr_b)[
                            :, k % 4, s_rel * P:(s_rel + 1) * P]
                        nc.tensor.matmul(
                            pv[:], lhsT=xt_sl, rhs=wv_t[:, k, :],
                            start=(k == 0), stop=(k == KCH - 1))
                    nc.vector.tensor_copy(v_r[4 * n + s_rel][:, :, 0:64], pv[:])

            def attn_block(n):
                nsl = slice(n * TB, (n + 1) * TB)
                for c in range(2):
                    Y = ps.tile([65, 2, TB], f32, tag="YP", name=f"Y{c}_{n}",
                                bufs=1)
                    n_k = 4 * n + 4
                    for k in range(n_k):
                        r = k - 4 * n
                        mt = P * r if r > 0 else 0
                        ksl = slice(k * P, (k + 1) * P)
                        qsl = slice(n * TB + mt, (n + 1) * TB)
                        S = ps.tile([P, 2, TB], f32, tag="SP", name=f"S{c}{n}_{k}",
                                    bufs=2)
                        nc.tensor.matmul(S[:, 0, mt:TB], lhsT=kT[c][0:64, ksl],
                                         rhs=qT[c][0:64, qsl], start=True, stop=True)
                        nc.tensor.matmul(S[:, 1, mt:TB], lhsT=kT[c][64:128, ksl],
                                         rhs=qT[c][64:128, qsl], start=True, stop=True)
                        e0 = tr.tile([P, 2, TB], bf16, tag="e0", name=f"e{c}{n}_{k}",
                                     bufs=3)
                        nc.scalar.activation(e0[:, :, mt:TB], S[:, :, mt:TB],
                                             Exp, scale=0.125)
                        if r >= 0:
                            em = tr.tile([P, 2, TB], bf16, tag="em",
                                         name=f"em{c}{n}_{k}", bufs=2)
                            nc.gpsimd.affine_select(
                                out=em[:, :, mt:TB], in_=e0[:, :, mt:TB],
                                pattern=[[0, 2], [1, TB - mt]],
                                compare_op=Alu.is_ge, fill=0.0,
                                base=mt - P * r, channel_multiplier=-1)
                            e0 = em
                        for h2 in range(2):
                            nc.tensor.matmul(
                                Y[:, h2, mt:TB],
                                lhsT=v_r[k][:, 2 * c + h2, :],
                                rhs=e0[:, h2, mt:TB],
                                start=(k == 0), stop=(k == n_k - 1))
                    yb = tr.tile([65, 2, TB], bf16, tag="yb", name=f"yb{c}{n}",
                                 bufs=2, padded_shape=[P, 2, TB])
                    nc.vector.tensor_copy(yb[:], Y[:])
                    for h2 in range(2):
                        nc.sync.dma_start(out=yT[c][64 * h2:64 * h2 + 64, nsl],
                                          in_=yb[0:64, h2, :])
                        nc.sync.dma_start(
                            out=den_bf[32 * (2 * c + h2):32 * (2 * c + h2) + 1, nsl],
                            in_=yb[64:65, h2, :])

            def outproj_block(n):
                nsl = slice(n * TB, (n + 1) * TB)
                den_f = tr.tile([P, TB], f32, tag="denf", name=f"denf{n}", bufs=2)
                nc.vector.tensor_copy(den_f[:], den_bf[:, nsl])
                nc.vector.reciprocal_approx_fast(out=invden_f[n][:], in_=den_f[:])
                nc.vector.tensor_copy(invden_b[n][:], invden_f[n][:])
                for c in range(2):
                    psi = ps.tile([P, TB], f32, tag="WK", name=f"psi{c}{n}",
                                  bufs=2)
                    nc.tensor.matmul(psi[:], lhsT=selpair_t[:, c * P:(c + 1) * P],
                                     rhs=invden_b[n][:], start=True, stop=True)
                    nc.vector.tensor_mul(yT[c][:, nsl], yT[c][:, nsl], psi[:])
                for o in range(4):
                    po = ps.tile([P, 2, TB], f32, tag="SP", name=f"po{o}_{n}",
                                 bufs=2)
                    for j in range(2):
                        osl = slice((2 * o + j) * P, (2 * o + j + 1) * P)
                        nc.tensor.matmul(po[:, j, :], lhsT=wp_t[:, 0, osl],
                                         rhs=yT[0][:, nsl], start=True, stop=False)
                        nc.tensor.matmul(po[:, j, :], lhsT=wp_t[:, 1, osl],
                                         rhs=yT[1][:, nsl], start=False, stop=True)
                    ob = tr.tile([P, 2, TB], f32, tag="ob", name=f"ob{o}_{n}", bufs=3)
                    nc.vector.tensor_copy(ob[:], po[:])
                    nc.sync.dma_start(out=out_d[:, o, :, nsl], in_=ob[:])

            # Software pipeline: emit P1(n+1) before attention(n) so the
            # tensor stream has ready projection work while block n's
            # repack DMAs land.
            p1_block(0)
            for n in range(NB):
                if n + 1 < NB:
                    p1_block(n + 1)
                attn_block(n)
                outproj_block(n)

    nc.compile()
    return nc


def _get_module():
    if "nc" not in _CACHE:
        _CACHE["nc"] = _build_module()
        _CACHE["consts"] = _build_consts()
    return _CACHE["nc"], _CACHE["consts"]


def _core_inputs(x, w_q, w_k, w_v, w_proj, core):
    import ml_dtypes
    bf = ml_dtypes.bfloat16
    b = core // 4
    g = core % 4
    heads = [4 * g + j for j in range(HPC)]

    xt = np.ascontiguousarray(x[b].T).reshape(KCH, P, T).transpose(1, 0, 2)
    xt = np.ascontiguousarray(xt).astype(bf)                # [128, 8, T]

    def chunked(a):
        # [C, F] -> [128, C//128, F] with chunk k = rows 128k..128k+127
        F = a.shape[1]
        return np.ascontiguousarray(
            a.reshape(a.shape[0] // P, P, F).transpose(1, 0, 2)).astype(bf)

    perm = np.empty(256, dtype=np.int64)
    for m in range(128):
        perm[m] = 64 * heads[m // 32] + (m % 32)             # x1 half
        perm[128 + m] = 64 * heads[m // 32] + 32 + (m % 32)  # x2 half
    wq = chunked(np.ascontiguousarray(w_q[perm, :].T))       # [128, 8, 256]
    wk = chunked(np.ascontiguousarray(w_k[perm, :].T))

    vperm = np.empty(256, dtype=np.int64)
    for m in range(256):
        vperm[m] = 64 * heads[m // 64] + (m % 64)
    wv = chunked(np.ascontiguousarray(w_v[vperm, :].T))      # [128, 8, 256]
    wp = chunked(np.ascontiguousarray(w_proj[:, vperm].T))   # [128, 2, C]
    return dict(xt=xt, wq=wq, wk=wk, wv=wv, wp=wp)


def kernel(x, w_q, w_k, w_v, w_proj, _trace=False, _trace_cores=None):
    from concourse.bass_utils import run_bass_kernel_spmd

    nc, consts = _get_module()
    x = np.asarray(x, dtype=np.float32)
    in_maps = []
    for core in range(N_CORES):
        m = _core_inputs(np.asarray(x), np.asarray(w_q), np.asarray(w_k),
                         np.asarray(w_v), np.asarray(w_proj), core)
        m.update(consts)
        in_maps.append(m)

    res = run_bass_kernel_spmd(nc, in_maps, list(range(N_CORES)),
                               trace=_trace, trace_cores=_trace_cores)
    outs = [res.results[c]["outT"] for c in range(N_CORES)]
    out = np.empty((B, T, C), dtype=np.float32)
    for b in range(B):
        acc = outs[4 * b].astype(np.float32)
        for g in range(1, 4):
            acc = acc + outs[4 * b + g]
        # acc [128, 4, 2, T]: orig row 256*o + 128*j + p at [p, o, j]
        acc = acc.transpose(1, 2, 0, 3).reshape(C, T)
        out[b] = acc.T
    if _trace:
        kernel._last_exec_time_ns = res.exec_time_ns
        kernel._last_results = res
    return out


# revision 19
# speedup vs baseline: 1.1770x; 1.1770x over previous
"""Causal self-attention (RMSNorm-QK + RoPE) Trainium2 Bass kernel, v2.

Problem: B=2, T=2048, C=1024, H=16 heads, D=64.
Sharding: 8 cores = 2 (batch) x 4 (head groups of 4 heads). Host sums the
4 column-parallel out-proj partials per batch and transposes.

v2 design (vs baseline ~360us):
- bf16 everywhere except PSUM accum, den/invden math, and the final output.
- Fully fused single pass over 4 token blocks of 512: projections -> RMS ->
  RoPE -> attention(j=n) -> normalize+out-proj(n). No serial phases.
- Scores matmuls packed 2 heads/instruction via K=64 row-tiling
  (base_partition 0/64 auto tile_position).
- exp: one ACTIVATE per (pair, key-block) over a [128,2,512] PSUM pair tile.
- One manual ACT table load (natural_log_exp_and_others) serves the RMS
  Ln/Exp rsqrt and the attention Exp: no table thrashing.
- Causal masks: 4 precomputed bf16 [128,2,512] tiles, applied with DVE mult.
- Denominator via ones-column in v (M=65 AV); 1/den via DVE
  reciprocal_approx_fast on f32; bf16 den storage.
- Single big weight DMA + 3D-AP x loads to avoid startup DMA serialization.
"""

import sys

for _p in ("/opt/trn_rl_repo",):
    if _p not in sys.path:
        sys.path.append(_p)

import numpy as np

B, T, C = 2, 2048, 1024
H_TOT, D = 16, 64
HPC = 4               # heads per core
N_CORES = 8
P = 128
NB = 4                # token blocks
TB = 512              # token block size
KCH = 8               # C / 128 contraction chunks
RMS_EPS = 1.1920928955078125e-07
ROPE_BASE = 10000.0
ACT_SET_LN_EXP = 6    # natural_log_exp_and_others in act_info.json

_CACHE = {}


def _build_consts():
    import ml_dtypes
    bf = ml_dtypes.bfloat16
    inv_freq = (1.0 / (ROPE_BASE ** (np.arange(0, D, 2, dtype=np.float32) / np.float32(D)))).astype(np.float32)
    pos = np.arange(T, dtype=np.float32)
    freqs = np.outer(pos, inv_freq).astype(np.float32)      # [T, 32]
    cos = np.cos(freqs).astype(np.float32)
    sin = np.sin(freqs).astype(np.float32)
    cosr = np.ascontiguousarray(np.tile(cos.T, (HPC, 1))).astype(bf)   # [128, T]
    sinr = np.ascontiguousarray(np.tile(sin.T, (HPC, 1))).astype(bf)
    # ind32 [128, 4]: per-32-row-group summing matrix (lhsT for RMS sums)
    ind32 = np.zeros((P, HPC), dtype=np.float32)
    for p_ in range(P):
        ind32[p_, p_ // 32] = 1.0
    # bc32 [36, 128]: broadcast inv (4 heads) to 32-row groups (lhsT),
    # replicated at rows 0:4 (q) and 32:36 (k)
    bc32 = np.zeros((36, P), dtype=np.float32)
    for p_ in range(P):
        bc32[p_ // 32, p_] = 1.0
        bc32[32 + p_ // 32, p_] = 1.0
    # selpair [128, 256]: chunk c: col m -> den row 32*(2c + m//64)
    selpair = np.zeros((P, 2 * P), dtype=np.float32)
    for c in range(2):
        for m in range(P):
            selpair[32 * (2 * c + m // 64), 128 * c + m] = 1.0
    return dict(cosr=cosr, sinr=sinr, ind32=ind32.astype(bf),
                bc32=bc32.astype(bf), selpair=selpair.astype(bf))


def _build_module():
    import concourse.bacc as bacc
    import concourse.mybir as mybir
    import concourse.tile as tile

    f32 = mybir.dt.float32
    bf16 = mybir.dt.bfloat16
    Exp = mybir.ActivationFunctionType.Exp
    Ln = mybir.ActivationFunctionType.Ln
    Alu = mybir.AluOpType

    nc = bacc.Bacc("TRN2", target_bir_lowering=False, debug=False,
                   num_devices=N_CORES)

    # DRAM tensors. x is [128, 8, T] (chunk k at [:, k, :], row p = chan 128k+p)
    xt_d = nc.dram_tensor("xt", [P, KCH, T], bf16, kind="ExternalInput").ap()
    wq_d = nc.dram_tensor("wq", [P, KCH, 256], bf16, kind="ExternalInput").ap()
    wk_d = nc.dram_tensor("wk", [P, KCH, 256], bf16, kind="ExternalInput").ap()
    wv_d = nc.dram_tensor("wv", [P, KCH, 256], bf16, kind="ExternalInput").ap()
    wp_d = nc.dram_tensor("wp", [P, 2, C], bf16, kind="ExternalInput").ap()
    cosr_d = nc.dram_tensor("cosr", [P, T], bf16, kind="ExternalInput").ap()
    sinr_d = nc.dram_tensor("sinr", [P, T], bf16, kind="ExternalInput").ap()
    ind32_d = nc.dram_tensor("ind32", [P, HPC], bf16, kind="ExternalInput").ap()
    bc32_d = nc.dram_tensor("bc32", [36, P], bf16, kind="ExternalInput").ap()
    selpair_d = nc.dram_tensor("selpair", [P, 2 * P], bf16, kind="ExternalInput").ap()
    # out rows permuted: [p, o, j] = orig row 256*o + 128*j + p (host undoes)
    out_d = nc.dram_tensor("outT", [P, 4, 2, T], f32, kind="ExternalOutput").ap()

    with tile.TileContext(nc) as tc:
        nc.scalar.add_instruction(mybir.InstLoadActFuncSet(
            name=nc.get_next_instruction_name(),
            act_func_set_id=ACT_SET_LN_EXP, ins=[], outs=[]))

        with (
            tc.tile_pool(name="sb", bufs=1) as sb,
            tc.tile_pool(name="tr", bufs=2) as tr,
            tc.tile_pool(name="ps", bufs=1, space="PSUM") as ps,
        ):
            # ---- persistent SBUF ----
            def load(name, dram, shape, dt=bf16):
                t = sb.tile(shape, dt, tag=name, name=name)
                nc.sync.dma_start(out=t[:], in_=dram)
                return t

            # load order = need order: wq/wk first, wp last
            wq_a = load("wq_a", wq_d[:, 0:4, :], [P, 4, 256])
            wq_b = load("wq_b", wq_d[:, 4:8, :], [P, 4, 256])
            wk_t = load("wk", wk_d[:, :, :], [P, KCH, 256])
            wv_t = load("wv", wv_d[:, :, :], [P, KCH, 256])
            ind32_t = load("ind32", ind32_d[:, :], [P, HPC])
            bc32_t = load("bc32", bc32_d[:, :], [36, P])
            cosr_t = load("cosr", cosr_d[:, :], [P, T])
            sinr_t = load("sinr", sinr_d[:, :], [P, T])
            selpair_t = load("selpair", selpair_d[:, :], [P, 2 * P])
            wp_t = load("wp", wp_d[:, :, :], [P, 2, C])

            qT = [sb.tile([P, T], bf16, tag=f"qT{c}", name=f"qT{c}")
                  for c in range(2)]
            kT = [sb.tile([P, T], bf16, tag=f"kT{c}", name=f"kT{c}")
                  for c in range(2)]
            v_r = [sb.tile([P, HPC, 65], bf16, tag=f"v{s}", name=f"v{s}")
                   for s in range(T // P)]
            for s in range(T // P):
                nc.gpsimd.memset(v_r[s][:, :, 64:65], 1.0)
            yT = [sb.tile([P, T], bf16, tag=f"yT{c}", name=f"yT{c}")
                  for c in range(2)]
            den_bf = sb.tile([P, T], bf16, tag="denbf", name="den_bf")
            nc.gpsimd.memset(den_bf[:], 1.0)

            eps_t = sb.tile([36, 1], f32, tag="epst", name="eps_t")
            nc.gpsimd.memset(eps_t[:], RMS_EPS)

            # masks: mask_r [128, 2, 512] bf16, 1 where q >= p + 128 r
            masks = []
            for r in range(4):
                m = sb.tile([P, 2, TB], bf16, tag=f"mask{r}", name=f"mask{r}")
                nc.gpsimd.memset(m[:], 1.0)
                nc.gpsimd.affine_select(
                    out=m[:], in_=m[:], pattern=[[0, 2], [1, TB]],
                    compare_op=Alu.is_ge, fill=0.0,
                    base=-P * r, channel_multiplier=-1)
                masks.append(m)

            invden_f = [sb.tile([P, TB], f32, tag="invdf", name=f"invdf{n}",
                                bufs=2) for n in range(NB)]
            invden_b = [sb.tile([P, TB], bf16, tag="invdb", name=f"invdb{n}",
                                bufs=2) for n in range(NB)]

            def rope_one(eng, x_pair, invb, rc1, rc2, nsl, nm):
                """rc1 = (x1 cos + x2 sin) inv ; rc2 = (x2 cos - x1 sin) inv."""
                x1 = x_pair[:, 0, :]
                x2 = x_pair[:, 1, :]
                ma = tr.tile([P, TB], bf16, tag=f"ma{nm}", name=f"ma{nm}", bufs=2)
                mb = tr.tile([P, TB], bf16, tag=f"mb{nm}", name=f"mb{nm}", bufs=2)
                eng.tensor_mul(ma[:], x1, cosr_t[:, nsl])
                eng.tensor_mul(mb[:], x2, sinr_t[:, nsl])
                eng.tensor_add(ma[:], ma[:], mb[:])
                eng.tensor_mul(rc1[:], ma[:], invb[:])
                mc = tr.tile([P, TB], bf16, tag=f"mc{nm}", name=f"mc{nm}", bufs=2)
                md = tr.tile([P, TB], bf16, tag=f"md{nm}", name=f"md{nm}", bufs=2)
                eng.tensor_mul(mc[:], x2, cosr_t[:, nsl])
                eng.tensor_mul(md[:], x1, sinr_t[:, nsl])
                eng.tensor_sub(mc[:], mc[:], md[:])
                eng.tensor_mul(rc2[:], mc[:], invb[:])

            def p1_block(n):
                """Projections + RMS inv + RoPE + repack + v for block n."""
                nsl = slice(n * TB, (n + 1) * TB)
                xr_a = tr.tile([P, 4, TB], bf16, tag="xra", name=f"xra{n}", bufs=2)
                nc.sync.dma_start(out=xr_a[:], in_=xt_d[:, 0:4, nsl])
                xr_b = tr.tile([P, 4, TB], bf16, tag="xrb", name=f"xrb{n}", bufs=2)
                nc.sync.dma_start(out=xr_b[:], in_=xt_d[:, 4:8, nsl])

                def xrc(k):
                    return (xr_a if k < 4 else xr_b)[:, k % 4, :]

                # ---- q/k projections + squares ----
                xps, sqs = {}, {}
                for tens, w_ts in (("q", (wq_a, wq_b)), ("k", (wk_t,))):
                    xp = tr.tile([P, 2, TB], bf16, tag="xp", name=f"xp{tens}{n}",
                                 bufs=2)
                    for half in range(2):
                        pg = ps.tile([P, TB], f32, tag="WK", name=f"p{tens}{half}_{n}",
                                     bufs=2)
                        for k in range(KCH):
                            if len(w_ts) == 2:
                                wsl = w_ts[k // 4][:, k % 4, 128 * half:128 * half + 128]
                            else:
                                wsl = w_ts[0][:, k, 128 * half:128 * half + 128]
                            nc.tensor.matmul(
                                pg[:], lhsT=wsl, rhs=xrc(k),
                                start=(k == 0), stop=(k == KCH - 1))
                        nc.vector.tensor_copy(xp[:, half, :], pg[:])
                    sq = tr.tile([P, 2, TB], bf16, tag="sq", name=f"sq{tens}{n}",
                                 bufs=2)
                    nc.vector.tensor_mul(sq[:], xp[:], xp[:])
                    xps[tens], sqs[tens] = xp, sq
                # ---- merged RMS inv: q at rows 0:4, k at rows 32:36 ----
                msp = ps.tile([36, TB], f32, tag="WK", name=f"ms{n}", bufs=2)
                for tens, rb in (("q", 0), ("k", 32)):
                    nc.tensor.matmul(msp[rb:rb + 4, :], lhsT=ind32_t[:],
                                     rhs=sqs[tens][:, 0, :], start=True, stop=False)
                    nc.tensor.matmul(msp[rb:rb + 4, :], lhsT=ind32_t[:],
                                     rhs=sqs[tens][:, 1, :], start=False, stop=True)
                invc = tr.tile([36, TB], bf16, tag="invc", name=f"invc{n}",
                               bufs=2)
                nc.scalar.activation(invc[:], msp[:], Ln,
                                     bias=eps_t[:], scale=1.0 / 64.0)
                nc.scalar.activation(invc[:], invc[:], Exp, scale=-0.5)
                # ---- RoPE + repack per tensor ----
                for tens, dstT, rb in (("q", qT, 0), ("k", kT, 32)):
                    invp = ps.tile([P, TB], f32, tag="WK", name=f"invp{tens}{n}",
                                   bufs=2)
                    nc.tensor.matmul(invp[:], lhsT=bc32_t[rb:rb + 4, :],
                                     rhs=invc[rb:rb + 4, :], start=True, stop=True)
                    invb = tr.tile([P, TB], bf16, tag="invb", name=f"invb{tens}{n}",
                                   bufs=2)
                    nc.vector.tensor_copy(invb[:], invp[:])
                    rc1 = tr.tile([P, TB], bf16, tag=f"rc1{tens}", name=f"rc1{tens}{n}",
                                  bufs=2)
                    rc2 = tr.tile([P, TB], bf16, tag=f"rc2{tens}", name=f"rc2{tens}{n}",
                                  bufs=2)
                    eng = nc.vector if tens == "q" else nc.gpsimd
                    rope_one(eng, xps[tens], invb, rc1, rc2, nsl, tens + str(n))
                    # repack into qT/kT: head h dims = rows 32h of rc1|rc2.
                    dma_eng = nc.gpsimd if tens == "q" else nc.sync
                    for c in range(2):
                        dst = dstT[c]
                        for h2, src in ((0, rc1), (1, rc2)):
                            dma_eng.dma_start(
                                out=dst[32 * h2:32 * h2 + 32, nsl],
                                in_=src[64 * c:64 * c + 32, :])
                            dma_eng.dma_start(
                                out=dst[64 + 32 * h2:64 + 32 * h2 + 32, nsl],
                                in_=src[64 * c + 32:64 * c + 64, :])

                # ---- v projection ----
                for s_rel in range(4):
                    pv = ps.tile([P, HPC, 64], f32, tag="WK", name=f"pv{n}_{s_rel}",
                                 bufs=2)
                    for k in range(KCH):
                        xt_sl = (xr_a if k < 4 else xr_b)[
                            :, k % 4, s_rel * P:(s_rel + 1) * P]
                        nc.tensor.matmul(
                            pv[:], lhsT=xt_sl, rhs=wv_t[:, k, :],
                            start=(k == 0), stop=(k == KCH - 1))
                    nc.vector.tensor_copy(v_r[4 * n + s_rel][:, :, 0:64], pv[:])

            def attn_block(n):
                nsl = slice(n * TB, (n + 1) * TB)
                for c in range(2):
                    pass
                with tc.high_priority():
                    _attn_body(n, nsl)

            def _attn_body(n, nsl):
                for c in range(2):
                    Y = ps.tile([65, 2, TB], f32, tag="YP", name=f"Y{c}_{n}",
                                bufs=1)
                    n_k = 4 * n + 4
                    for k in range(n_k):
                        r = k - 4 * n
                        mt = P * r if r > 0 else 0
                        ksl = slice(k * P, (k + 1) * P)
                        qsl = slice(n * TB + mt, (n + 1) * TB)
                        S = ps.tile([P, 2, TB], f32, tag="SP", name=f"S{c}{n}_{k}",
                                    bufs=2)
                        nc.tensor.matmul(S[:, 0, mt:TB], lhsT=kT[c][0:64, ksl],
                                         rhs=qT[c][0:64, qsl], start=True, stop=True)
                        nc.tensor.matmul(S[:, 1, mt:TB], lhsT=kT[c][64:128, ksl],
                                         rhs=qT[c][64:128, qsl], start=True, stop=True)
                        e0 = tr.tile([P, 2, TB], bf16, tag="e0", name=f"e{c}{n}_{k}",
                                     bufs=3)
                        nc.scalar.activation(e0[:, :, mt:TB], S[:, :, mt:TB],
                                             Exp, scale=0.125)
                        if r >= 0:
                            em = tr.tile([P, 2, TB], bf16, tag="em",
                                         name=f"em{c}{n}_{k}", bufs=3)
                            nc.vector.tensor_mul(em[:, :, mt:TB], e0[:, :, mt:TB],
                                                 masks[r][:, :, mt:TB])
                            e0 = em
                        for h2 in range(2):
                            nc.tensor.matmul(
                                Y[:, h2, mt:TB],
                                lhsT=v_r[k][:, 2 * c + h2, :],
                                rhs=e0[:, h2, mt:TB],
                                start=(k == 0), stop=(k == n_k - 1))
                    yb = tr.tile([65, 2, TB], bf16, tag="yb", name=f"yb{c}{n}",
                                 bufs=2, padded_shape=[P, 2, TB])
                    nc.vector.tensor_copy(yb[:], Y[:])
                    for h2 in range(2):
                        nc.sync.dma_start(out=yT[c][64 * h2:64 * h2 + 64, nsl],
                                          in_=yb[0:64, h2, :])
                        nc.sync.dma_start(
                            out=den_bf[32 * (2 * c + h2):32 * (2 * c + h2) + 1, nsl],
                            in_=yb[64:65, h2, :])

            def outproj_block(n):
                nsl = slice(n * TB, (n + 1) * TB)
                den_f = tr.tile([P, TB], f32, tag="denf", name=f"denf{n}", bufs=2)
                nc.scalar.copy(den_f[:], den_bf[:, nsl])
                nc.vector.reciprocal_approx_fast(out=invden_f[n][:], in_=den_f[:])
                nc.vector.tensor_copy(invden_b[n][:], invden_f[n][:])
                for c in range(2):
                    psi = ps.tile([P, TB], f32, tag="WK", name=f"psi{c}{n}",
                                  bufs=2)
                    nc.tensor.matmul(psi[:], lhsT=selpair_t[:, c * P:(c + 1) * P],
                                     rhs=invden_b[n][:], start=True, stop=True)
                    nc.vector.tensor_mul(yT[c][:, nsl], yT[c][:, nsl], psi[:])
                for o in range(4):
                    po = ps.tile([P, 2, TB], f32, tag="SP", name=f"po{o}_{n}",
                                 bufs=2)
                    for j in range(2):
                        osl = slice((2 * o + j) * P, (2 * o + j + 1) * P)
                        nc.tensor.matmul(po[:, j, :], lhsT=wp_t[:, 0, osl],
                                         rhs=yT[0][:, nsl], start=True, stop=False)
                        nc.tensor.matmul(po[:, j, :], lhsT=wp_t[:, 1, osl],
                                         rhs=yT[1][:, nsl], start=False, stop=True)
                    ob = tr.tile([P, 2, TB], f32, tag="ob", name=f"ob{o}_{n}", bufs=3)
                    nc.vector.tensor_copy(ob[:], po[:])
                    nc.sync.dma_start(out=out_d[:, o, :, nsl], in_=ob[:])

            # Software pipeline: emit P1(n+1) before attention(n) so the
            # tensor stream has ready projection work while block n's
            # repack DMAs land.
            p1_block(0)
            for n in range(NB):
                if n + 1 < NB:
                    p1_block(n + 1)
                attn_block(n)
                outproj_block(n)

    nc.compile()
    return nc


def _get_module():
    if "nc" not in _CACHE:
        _CACHE["nc"] = _build_module()
        _CACHE["consts"] = _build_consts()
    return _CACHE["nc"], _CACHE["consts"]


def _core_inputs(x, w_q, w_k, w_v, w_proj, core):
    import ml_dtypes
    bf = ml_dtypes.bfloat16
    b = core // 4
    g = core % 4
    heads = [4 * g + j for j in range(HPC)]

    xt = np.ascontiguousarray(x[b].T).reshape(KCH, P, T).transpose(1, 0, 2)
    xt = np.ascontiguousarray(xt).astype(bf)                # [128, 8, T]

    def chunked(a):
        # [C, F] -> [128, C//128, F] with chunk k = rows 128k..128k+127
        F = a.shape[1]
        return np.ascontiguousarray(
            a.reshape(a.shape[0] // P, P, F).transpose(1, 0, 2)).astype(bf)

    perm = np.empty(256, dtype=np.int64)
    for m in range(128):
        perm[m] = 64 * heads[m // 32] + (m % 32)             # x1 half
        perm[128 + m] = 64 * heads[m // 32] + 32 + (m % 32)  # x2 half
    wq = chunked(np.ascontiguousarray(w_q[perm, :].T))       # [128, 8, 256]
    wk = chunked(np.ascontiguousarray(w_k[perm, :].T))

    vperm = np.empty(256, dtype=np.int64)
    for m in range(256):
        vperm[m] = 64 * heads[m // 64] + (m % 64)
    wv = chunked(np.ascontiguousarray(w_v[vperm, :].T))      # [128, 8, 256]
    wp = chunked(np.ascontiguousarray(w_proj[:, vperm].T))   # [128, 2, C]
    return dict(xt=xt, wq=wq, wk=wk, wv=wv, wp=wp)


def kernel(x, w_q, w_k, w_v, w_proj, _trace=False, _trace_cores=None):
    from concourse.bass_utils import run_bass_kernel_spmd

    nc, consts = _get_module()
    x = np.asarray(x, dtype=np.float32)
    in_maps = []
    for core in range(N_CORES):
        m = _core_inputs(np.asarray(x), np.asarray(w_q), np.asarray(w_k),
                         np.asarray(w_v), np.asarray(w_proj), core)
        m.update(consts)
        in_maps.append(m)

    res = run_bass_kernel_spmd(nc, in_maps, list(range(N_CORES)),
                               trace=_trace, trace_cores=_trace_cores)
    outs = [res.results[c]["outT"] for c in range(N_CORES)]
    out = np.empty((B, T, C), dtype=np.float32)
    for b in range(B):
        acc = outs[4 * b].astype(np.float32)
        for g in range(1, 4):
            acc = acc + outs[4 * b + g]
        # acc [128, 4, 2, T]: orig row 256*o + 128*j + p at [p, o, j]
        acc = acc.transpose(1, 2, 0, 3).reshape(C, T)
        out[b] = acc.T
    if _trace:
        kernel._last_exec_time_ns = res.exec_time_ns
        kernel._last_results = res
    return out
